# revision 39
# baseline (speedup 1.0000x reference)
"""Trainium2 Bass kernel for nn_AttentionBlock (GNN message passing).

Contract: kernel(**inputs) takes the FULL (unsharded) inputs
    x         [50000, 512] f32
    edge_index[2, 800000]  int64
    W_qkv     [1536, 512]  f32
    b_qkv     [1536]       f32  (zeros)
    W_ff      [512, 512]   f32
    b_ff      [512]        f32  (zeros)
and returns the FULL output [50000, 512] f32, computed on 8
NeuronCores.

Sharding: edges are sharded BY RECEIVER RANGE — core c owns receivers
[c*6250, (c+1)*6250), so the segment softmax and the scattered
V-aggregation for a given receiver live entirely on one core and no
cross-core reduction is needed.  Node features and weights are
replicated (each core redundantly computes K/V for all nodes, Q for
its local nodes only).

v7 (HW-validated 1.79 ms vs 3.60 ms baseline; rel err 7.0e-4 vs the
2e-2 gate): fp16 hot path, engine-balanced phase 2:
  Phase 1: qkv = x @ W_qkv.T with x/W rounded to fp16 — single-pass
    matmuls at full PE rate.  Each core computes K|V only for nodes
    that appear as senders in its edge shard (compaction, ~12% less
    work) -> fp16 kv_table (K head-major, V depth-major), local Q
    rows -> fp16 q_table.
  Phase 2: edges sorted by local receiver, grouped into 49 blocks of
    128 receivers; per-block tile count is variable (max over cores,
    same program on all 8).  Per macro (<=6 tiles of 128 edges):
    per-tile indirect-DMA gathers of KV[sender] rows (offset [P,1] —
    batched offsets return garbage on real HW); one-hot S streamed
    from the host (partition-major contiguous); S^T built as
    (slot_id[p] == rT[e]) via tensor_scalar on DVE (4x mode), where
    rT is a DMA partition-broadcast of the per-edge slot row;
    Q broadcast to edges via PE (S^T @ q_blk); ScalarE downcasts qx
    to fp16 in pairs; DVE qk mult / pairwise-halve / per-head reduce
    / attv mult (halve works around TensorReduce's 1x-only mode; V
    depth-major makes the ex broadcast innermost-contiguous for 2x);
    segment sums via PE matmuls accumulated in PSUM over the block:
    msg += S^T@attv, den += S^T@ex.  exp() without segment-max:
    unit-variance scores are O(+-6), max ex ~350 << fp16 max 65504.
  Epilogue per block: msg/(den+1e-30) -> fp16 -> PE transpose ->
    out_block = msg_norm @ W_ff.T (fp16 matmuls, fp32 accumulate).
"""

import sys

sys.path.insert(0, "/opt/trn_rl_repo")

from dataclasses import dataclass, field

import numpy as np

import concourse.bass as bass
import concourse.bacc as bacc
import concourse.mybir as mybir
import concourse.tile as tile

F32 = mybir.dt.float32
F16 = mybir.dt.float16
I32 = mybir.dt.int32
AX = mybir.AxisListType
OP = mybir.AluOpType
ACTF = mybir.ActivationFunctionType

P = 128


@dataclass(frozen=True)
class Cfg:
    N: int
    L: int
    D: int
    H: int
    DK: int
    DV: int
    tb: tuple  # tiles per receiver-block (shared across cores)
    NT: int = 0  # node tiles after per-core sender compaction (0 = full)
    n_cores: int = 8

    @property
    def CD(self):
        assert self.D % P == 0
        return self.D // P

    @property
    def QC(self):
        return self.H * self.DK

    @property
    def KVC(self):
        return self.H * (self.DK + self.DV)

    @property
    def N_pad(self):
        if self.NT:
            return self.NT * P
        return ((self.N + P - 1) // P) * P

    @property
    def n_node_tiles(self):
        return self.N_pad // P

    @property
    def n_blocks(self):
        return (self.L + P - 1) // P

    @property
    def L_pad(self):
        return self.n_blocks * P

    @property
    def n_local_tiles(self):
        return self.n_blocks

    @property
    def T_total(self):
        return int(sum(self.tb))

    @property
    def block_off(self):
        off = [0]
        for t in self.tb:
            off.append(off[-1] + t)
        return off


def macro_sizes_of(t):
    """Split t tiles into balanced macros of size <= 6 (e.g. 17 -> [6,6,5])."""
    n = (t + 5) // 6
    base, rem = divmod(t, n)
    return [base + 1] * rem + [base] * (n - rem)


def prep_xt_tiles(x_pad, cfg):
    nt = cfg.n_node_tiles
    b = x_pad.reshape(nt, P, cfg.CD, P)
    b = np.ascontiguousarray(b.transpose(0, 3, 2, 1))
    return b.astype(np.float16)


def prep_w_chunks(wT, out_cols):
    D, C = wT.shape
    assert C == out_cols
    return np.ascontiguousarray(wT.reshape(D // P, P, C).transpose(1, 0, 2))


def host_prep(x, edge_index, W_qkv, b_qkv, W_ff, b_ff, cfg):
    N, L, D = cfg.N, cfg.L, cfg.D
    H, DK, DV = cfg.H, cfg.DK, cfg.DV
    assert not np.any(b_qkv), "b_qkv must be zero (fast path)"
    assert not np.any(b_ff), "b_ff must be zero (fast path)"

    senders = np.asarray(edge_index[0], dtype=np.int64)
    receivers = np.asarray(edge_index[1], dtype=np.int64)

    # W_qkv^T with the V output-columns permuted to depth-major (d,h)
    # so attv's ex-broadcast is innermost-contiguous on device.
    WqkvT = np.ascontiguousarray(W_qkv.T.astype(np.float32))  # [D, 1536]
    v_cols = WqkvT[:, 2 * H * DK:].reshape(D, H, DV)
    v_perm = np.ascontiguousarray(v_cols.transpose(0, 2, 1)).reshape(D, H * DV)
    Wq_perm = np.concatenate([WqkvT[:, :2 * H * DK], v_perm], axis=1)
    w_t = prep_w_chunks(Wq_perm, Wq_perm.shape[1]).astype(np.float16)

    # W_ff^T rows permuted to match msg's (d,h) column order.
    WffT = np.ascontiguousarray(W_ff.T.astype(np.float32))  # [H*DV, D]
    WffT_perm = np.ascontiguousarray(
        WffT.reshape(H, DV, D).transpose(1, 0, 2)).reshape(H * DV, D)
    wff = prep_w_chunks(WffT_perm, D).astype(np.float16)

    ident = np.eye(P, dtype=np.float16)
    iotac = np.arange(P, dtype=np.float32)[:, None].copy()

    T_total = cfg.T_total
    off = cfg.block_off

    in_maps, metas = [], []
    for c in range(cfg.n_cores):
        base = c * L
        x_rot = np.roll(x, -base, axis=0)

        mask = (receivers >= base) & (receivers < base + L)
        r_loc = (receivers[mask] - base).astype(np.int64)
        s_rot = ((senders[mask] - base) % N).astype(np.int64)
        order = np.argsort(r_loc, kind="stable")
        r_loc = r_loc[order]
        s_rot = s_rot[order]

        # compact: local nodes [0,L) first (need Q), then only the
        # non-local nodes that actually appear as senders (need K/V)
        nl_used = np.unique(s_rot[s_rot >= L])
        s_rot = np.where(s_rot < L, s_rot,
                         L + np.searchsorted(nl_used, s_rot))
        n_used = L + len(nl_used)
        assert n_used <= cfg.N_pad, f"core {c}: {n_used} > {cfg.N_pad}"
        x_pad = np.zeros((cfg.N_pad, D), np.float32)
        x_pad[:L] = x_rot[:L]
        x_pad[L:n_used] = x_rot[nl_used]
        xt = prep_xt_tiles(x_pad, cfg)

        blk = r_loc // P
        s_idx = np.zeros((P, T_total), np.int32)
        r_f = np.full((P, T_total), -1.0, np.float16)
        for b in range(cfg.n_blocks):
            sel = blk == b
            eb_s = s_rot[sel]
            eb_r = r_loc[sel] % P  # within-block receiver slot
            so = np.argsort(eb_s, kind="stable")
            eb_s = eb_s[so]
            eb_r = eb_r[so]
            ne = len(eb_s)
            cap = cfg.tb[b] * P
            assert ne <= cap, f"core {c} block {b}: {ne} edges > cap {cap}"
            t0 = off[b]
            full = np.zeros(cap, np.int64)
            full[:ne] = eb_s
            s_idx[:, t0:t0 + cfg.tb[b]] = full.reshape(cfg.tb[b], P).T
            fullf = np.full(cap, -1.0, np.float16)
            fullf[:ne] = eb_r.astype(np.float16)
            r_f[:, t0:t0 + cfg.tb[b]] = fullf.reshape(cfg.tb[b], P).T

        # one-hot S, partition-major: s_oh[e, t, r] = (r_loc[e,t] == r)
        rf_i = r_f.astype(np.float32).astype(np.int64)  # [P, T_total]
        s_oh = (rf_i[:, :, None] ==
                np.arange(P, dtype=np.int64)[None, None, :]
                ).astype(np.float16)
        r_fT = np.ascontiguousarray(r_f.T)  # [T_total, P]

        in_maps.append({
            "xt": xt,
            "w_t": w_t, "wff": wff,
            "s_idx": s_idx, "s_oh": s_oh, "r_fT": r_fT,
            "iotac": iotac, "ident": ident,
        })
        metas.append({})
    return in_maps, metas


def build_nc(cfg, num_devices=1):
    N_pad, D, H, DK, DV = cfg.N_pad, cfg.D, cfg.H, cfg.DK, cfg.DV
    CD, QC, KVC = cfg.CD, cfg.QC, cfg.KVC
    C = QC + KVC
    scale = 1.0 / np.sqrt(DK)

    nc = bacc.Bacc("TRN2", target_bir_lowering=False, debug=False,
                   num_devices=num_devices)

    xt_d = nc.dram_tensor("xt", [cfg.n_node_tiles, P, CD, P], F16,
                          kind="ExternalInput")
    w_t_d = nc.dram_tensor("w_t", [P, CD, C], F16, kind="ExternalInput")
    wff_d = nc.dram_tensor("wff", [P, (H * DV) // P, D], F16,
                           kind="ExternalInput")
    s_idx_d = nc.dram_tensor("s_idx", [P, cfg.T_total], I32,
                             kind="ExternalInput")
    s_oh_d = nc.dram_tensor("s_oh", [P, cfg.T_total, P], F16,
                            kind="ExternalInput")
    r_fT_d = nc.dram_tensor("r_fT", [cfg.T_total, P], F16,
                            kind="ExternalInput")
    iotac_d = nc.dram_tensor("iotac", [P, 1], F32, kind="ExternalInput")
    ident_d = nc.dram_tensor("ident", [P, P], F16, kind="ExternalInput")

    out_d = nc.dram_tensor("out", [cfg.L, D], F32, kind="ExternalOutput")

    kv_table = nc.dram_tensor("kv_table", [N_pad, KVC], F16)
    q_table = nc.dram_tensor("q_table", [cfg.L_pad, QC], F16)

    kv_col_chunks = [(i, min(512, KVC - i)) for i in range(0, KVC, 512)]
    off = cfg.block_off

    with tile.TileContext(nc) as tc:
        with tc.tile_pool(name="const", bufs=1) as cpool:
            wff_t = cpool.tile([P, (H * DV) // P, D], F16)
            s_idx_t = cpool.tile([P, cfg.T_total], I32)
            iotac_t = cpool.tile([P, 1], F32)
            ident_t = cpool.tile([P, P], F16)
            nc.sync.dma_start(out=ident_t[:], in_=ident_d[:])
            nc.sync.dma_start(out=iotac_t[:], in_=iotac_d[:])
            nc.sync.dma_start(out=wff_t[:], in_=wff_d[:])
            nc.sync.dma_start(out=s_idx_t[:], in_=s_idx_d[:])

            # ---- Phase 1: qkv projection, fp16 tables ----
            with tc.tile_pool(name="p1w", bufs=1) as wp, \
                 tc.tile_pool(name="p1sb", bufs=5) as sb, \
                 tc.tile_pool(name="p1ps", bufs=2, space="PSUM") as ps:
                w_t_t = wp.tile([P, CD, C], F16)
                nc.sync.dma_start(out=w_t_t[:], in_=w_t_d[:])
                nnt = cfg.n_node_tiles
                for nt0 in range(0, nnt, 2):
                    npair = min(2, nnt - nt0)
                    xh2 = sb.tile([P, 2, CD, P], F16, tag="xh")
                    nc.sync.dma_start(
                        out=xh2[:, :npair, :, :],
                        in_=xt_d[nt0:nt0 + npair].rearrange(
                            "t p c k -> p t c k"))
                    for j in range(npair):
                        nt = nt0 + j
                        xh = xh2[:, j]

                        def do_cols(dst_ps, col0, ncols):
                            for cch in range(CD):
                                nc.tensor.matmul(
                                    out=dst_ps[:, :ncols],
                                    lhsT=xh[:, cch, :],
                                    rhs=w_t_t[:, cch, col0:col0 + ncols],
                                    start=(cch == 0), stop=(cch == CD - 1))

                        kv_sb = sb.tile([P, KVC], F16, tag="kvsb")
                        for ci, (c0, cn) in enumerate(kv_col_chunks):
                            kv_ps = ps.tile([P, cn], F32, tag=f"kvps{ci}")
                            do_cols(kv_ps, QC + c0, cn)
                            if ci % 2 == 0:
                                nc.scalar.copy(out=kv_sb[:, c0:c0 + cn],
                                               in_=kv_ps[:, :cn])
                            else:
                                nc.vector.tensor_copy(
                                    out=kv_sb[:, c0:c0 + cn],
                                    in_=kv_ps[:, :cn])
                        nc.sync.dma_start(
                            out=kv_table[nt * P:(nt + 1) * P, :],
                            in_=kv_sb[:])

                        if nt < cfg.n_local_tiles:
                            q_sb = sb.tile([P, QC], F16, tag="qsb")
                            q_ps = ps.tile([P, QC], F32, tag="qps")
                            do_cols(q_ps, 0, QC)
                            nc.scalar.copy(out=q_sb[:], in_=q_ps[:])
                            nc.sync.dma_start(
                                out=q_table[nt * P:(nt + 1) * P, :],
                                in_=q_sb[:])

            # ---- Phase 2: per-block edge attention ----
            with tc.tile_pool(name="p2sb", bufs=5) as sb, \
                 tc.tile_pool(name="p2sb1", bufs=2) as sb1, \
                 tc.tile_pool(name="p2ps", bufs=2, space="PSUM") as ps, \
                 tc.tile_pool(name="p2ps1", bufs=1, space="PSUM") as ps1:
                for b in range(cfg.n_blocks):
                    Tb = cfg.tb[b]
                    msg_ps = ps1.tile([P, H * DV], F32, tag="msg")
                    den_ps = ps1.tile([P, H], F32, tag="den")
                    q_blk = sb1.tile([P, QC], F16, tag="qblk")
                    nc.sync.dma_start(out=q_blk[:],
                                      in_=q_table[b * P:(b + 1) * P, :])
                    gt = off[b]
                    # per-edge receiver slot, broadcast to all partitions
                    rT_blk = sb1.tile([P, max(cfg.tb), P], F16, tag="rT")
                    nc.sync.dma_start(
                        out=rT_blk[:, :Tb, :],
                        in_=r_fT_d[gt:gt + Tb, :][None, :, :]
                            .to_broadcast([P, Tb, P]))
                    ti = 0
                    for msz in macro_sizes_of(Tb):
                        t0 = gt + ti
                        kvg = sb.tile([P, msz, KVC], F16, tag="kvg")
                        for k in range(msz):
                            nc.gpsimd.indirect_dma_start(
                                out=kvg[:, k, :], out_offset=None,
                                in_=kv_table[:],
                                in_offset=bass.IndirectOffsetOnAxis(
                                    ap=s_idx_t[:, t0 + k:t0 + k + 1],
                                    axis=0))
                        # host-precomputed one-hot S for these tiles
                        S_m = sb.tile([P, msz, P], F16, tag="soh")
                        nc.sync.dma_start(
                            out=S_m[:],
                            in_=s_oh_d[:, t0:t0 + msz, :])
                        # S^T[j, e] = (j == slot[e]) via tensor_scalar (4x)
                        st_sb = sb.tile([P, msz, P], F16, tag="stsb")
                        nc.vector.tensor_scalar(
                            out=st_sb[:], in0=rT_blk[:, ti:ti + msz, :],
                            scalar1=iotac_t[:, :1], scalar2=None,
                            op0=OP.is_equal)
                        qx_sb = sb.tile([P, msz, QC], F16, tag="qx")
                        for k0 in range(0, msz, 2):
                            kp = min(2, msz - k0)
                            qx_ps = ps.tile([P, 2, QC], F32, tag="qxps")
                            for j in range(kp):
                                nc.tensor.matmul(
                                    out=qx_ps[:, j, :],
                                    lhsT=st_sb[:, k0 + j, :],
                                    rhs=q_blk[:], start=True, stop=True)
                            nc.scalar.copy(out=qx_sb[:, k0:k0 + kp, :],
                                           in_=qx_ps[:, :kp, :])
                        qk = sb.tile([P, msz, QC], F16, tag="qk")
                        nc.vector.tensor_tensor(
                            out=qk[:], in0=qx_sb[:], in1=kvg[:, :, :QC],
                            op=OP.mult)
                        # halve twice at 2x (second in place), then a
                        # 16-wide 1x reduce (TensorReduce has no 2x mode)
                        qkv_v = qk[:].rearrange("p m (h t d) -> p m h t d",
                                                h=H, t=2)
                        qh = sb.tile([P, msz, H, DK // 2], F16, tag="qh")
                        nc.vector.tensor_tensor(
                            out=qh[:], in0=qkv_v[:, :, :, 0, :],
                            in1=qkv_v[:, :, :, 1, :], op=OP.add)
                        nc.vector.tensor_tensor(
                            out=qh[:, :, :, :DK // 4],
                            in0=qh[:, :, :, :DK // 4],
                            in1=qh[:, :, :, DK // 4:], op=OP.add)
                        sc = sb.tile([P, msz, H], F16, tag="sc")
                        with nc.allow_low_precision(reason="fp16 scores"):
                            nc.vector.tensor_reduce(
                                out=sc[:], in_=qh[:, :, :, :DK // 4],
                                axis=AX.X, op=OP.add)
                        ex = sb.tile([P, msz, H], F16, tag="ex")
                        nc.scalar.activation(out=ex[:], in_=sc[:],
                                             func=ACTF.Exp, scale=scale)
                        # attv[e, d, h] = V[e, d, h] * ex[e, h]
                        attv = sb.tile([P, msz, H * DV], F16, tag="attv")
                        nc.vector.tensor_tensor(
                            out=attv[:].rearrange("p m (d h) -> p m d h",
                                                  h=H),
                            in0=kvg[:, :, QC:].rearrange(
                                "p m (d h) -> p m d h", h=H),
                            in1=ex[:, :, None, :].to_broadcast(
                                [P, msz, DV, H]),
                            op=OP.mult)
                        for k in range(msz):
                            t = ti + k
                            nc.tensor.matmul(
                                out=msg_ps[:], lhsT=S_m[:, k, :],
                                rhs=attv[:, k, :],
                                start=(t == 0), stop=(t == Tb - 1))
                            nc.tensor.matmul(
                                out=den_ps[:], lhsT=S_m[:, k, :],
                                rhs=ex[:, k, :],
                                start=(t == 0), stop=(t == Tb - 1))
                        ti += msz

                    den_sb = sb1.tile([P, H], F32, tag="densb")
                    nc.vector.tensor_scalar_add(out=den_sb[:], in0=den_ps[:],
                                                scalar1=1e-30)
                    rec = sb1.tile([P, H], F32, tag="rec")
                    nc.vector.reciprocal(out=rec[:], in_=den_sb[:])
                    msgn = sb1.tile([P, H * DV], F16, tag="msgn")
                    nc.vector.tensor_tensor(
                        out=msgn[:].rearrange("p (d h) -> p d h", h=H),
                        in0=msg_ps[:].rearrange("p (d h) -> p d h", h=H),
                        in1=rec[:, None, :].to_broadcast([P, DV, H]),
                        op=OP.mult)
                    hdv = H * DV
                    n_tch = hdv // P
                    mT_ps = ps1.tile([P, n_tch, P], F16, tag="mT")
                    for cch in range(n_tch):
                        nc.tensor.transpose(
                            out=mT_ps[:, cch, :],
                            in_=msgn[:, cch * P:(cch + 1) * P],
                            identity=ident_t[:])
                    mT_sb = sb1.tile([P, n_tch, P], F16, tag="mTsb")
                    nc.scalar.copy(out=mT_sb[:], in_=mT_ps[:])
                    out_ps = ps1.tile([P, D], F32, tag="outps")
                    for cch in range(n_tch):
                        nc.tensor.matmul(
                            out=out_ps[:],
                            lhsT=mT_sb[:, cch, :],
                            rhs=wff_t[:, cch, :],
                            start=(cch == 0), stop=(cch == n_tch - 1))
                    out_sb = sb1.tile([P, D], F32, tag="outsb")
                    nc.scalar.copy(out=out_sb[:], in_=out_ps[:])
                    r0 = b * P
                    nrow = min(P, cfg.L - r0)
                    nc.sync.dma_start(out=out_d[r0:r0 + nrow, :],
                                      in_=out_sb[:nrow, :])

    nc.compile()
    return nc


def _derive_tb(edge_index, N, L, n_cores):
    r = np.asarray(edge_index[1], dtype=np.int64)
    n_blocks = (L + P - 1) // P
    counts = np.zeros((n_cores, n_blocks), np.int64)
    for c in range(n_cores):
        m = (r >= c * L) & (r < (c + 1) * L)
        rl = r[m] - c * L
        counts[c] = np.bincount(rl // P, minlength=n_blocks)
    tb = np.maximum(1, np.ceil(counts.max(axis=0) / P).astype(int))
    return tuple(int(t) for t in tb)


def make_cfg(x, edge_index, n_cores=8):
    N, D = x.shape
    H = 8
    DV = DK = 64
    assert N % n_cores == 0
    L = N // n_cores
    tb = _derive_tb(edge_index, N, L, n_cores)
    s = np.asarray(edge_index[0], dtype=np.int64)
    r = np.asarray(edge_index[1], dtype=np.int64)
    NT = 0
    for c in range(n_cores):
        base = c * L
        m = (r >= base) & (r < base + L)
        s_rot = (s[m] - base) % N
        n_used = L + len(np.unique(s_rot[s_rot >= L]))
        NT = max(NT, (n_used + P - 1) // P)
    return Cfg(N=N, L=L, D=D, H=H, DK=DK, DV=DV, tb=tb, NT=NT,
               n_cores=n_cores)


_CACHE = {}


def _get_runner(cfg):
    """Build nc + reusable jitted SPMD callable (cached per config)."""
    key = cfg
    if key in _CACHE:
        return _CACHE[key]

    import jax
    from jax.sharding import Mesh, PartitionSpec
    from jax.experimental.shard_map import shard_map
    from concourse import bass2jax
    from concourse.bass2jax import _bass_exec_p, install_neuronx_cc_hook

    nc = build_nc(cfg, num_devices=cfg.n_cores)

    install_neuronx_cc_hook()
    partition_name = (nc.partition_id_tensor.name
                      if nc.partition_id_tensor else None)
    in_names, out_names, out_avals, zero_outs = [], [], [], []
    for alloc in nc.m.functions[0].allocations:
        if not isinstance(alloc, mybir.MemoryLocationSet):
            continue
        name = alloc.memorylocations[0].name
        if alloc.kind == "ExternalInput":
            if name != partition_name:
                in_names.append(name)
        elif alloc.kind == "ExternalOutput":
            out_names.append(name)
            shape = tuple(alloc.tensor_shape)
            dtype = mybir.dt.np(alloc.dtype)
            out_avals.append(jax.core.ShapedArray(shape, dtype))
            zero_outs.append(np.zeros(shape, dtype))
    n_params = len(in_names)
    all_in_names = list(in_names) + list(out_names)
    if partition_name is not None:
        all_in_names.append(partition_name)

    def _body(*args):
        operands = list(args)
        if partition_name is not None:
            operands.append(bass2jax.partition_id_tensor())
        outs = _bass_exec_p.bind(
            *operands,
            out_avals=tuple(out_avals),
            in_names=tuple(all_in_names),
            out_names=tuple(out_names),
            lowering_input_output_aliases=(),
            sim_require_finite=True,
            sim_require_nnan=True,
            nc=nc,
        )
        return tuple(outs)

    devices = jax.devices()[:cfg.n_cores]
    mesh = Mesh(np.asarray(devices), ("core",))
    in_specs = (PartitionSpec("core"),) * (n_params + len(out_names))
    out_specs = (PartitionSpec("core"),) * len(out_names)
    fn = jax.jit(
        shard_map(_body, mesh=mesh, in_specs=in_specs,
                  out_specs=out_specs, check_rep=False),
        keep_unused=True,
    )
    sharding = jax.sharding.NamedSharding(mesh, PartitionSpec("core"))

    def run(in_maps):
        args = []
        for name in in_names:
            cat = np.concatenate(
                [np.asarray(m[name]) for m in in_maps], axis=0)
            args.append(jax.device_put(cat, sharding))
        for z in zero_outs:
            args.append(jax.device_put(
                np.zeros((cfg.n_cores * z.shape[0], *z.shape[1:]), z.dtype),
                sharding))
        out_arrs = fn(*args)
        jax.block_until_ready(out_arrs)
        oi = out_names.index("out")
        full = np.asarray(out_arrs[oi]).reshape(
            cfg.n_cores, *out_avals[oi].shape)
        return full

    _CACHE[key] = (nc, fn, run, sharding, in_names, out_names, out_avals,
                   zero_outs)
    return _CACHE[key]


def kernel(x, edge_index, W_qkv, b_qkv, W_ff, b_ff):
    x = np.asarray(x, dtype=np.float32)
    edge_index = np.asarray(edge_index)
    W_qkv = np.asarray(W_qkv, dtype=np.float32)
    b_qkv = np.asarray(b_qkv, dtype=np.float32)
    W_ff = np.asarray(W_ff, dtype=np.float32)
    b_ff = np.asarray(b_ff, dtype=np.float32)

    cfg = make_cfg(x, edge_index)
    in_maps, _ = host_prep(x, edge_index, W_qkv, b_qkv, W_ff, b_ff, cfg)
    _, _, run, *_rest = _get_runner(cfg)
    full = run(in_maps)  # [n_cores, L, D]
    return np.ascontiguousarray(full.reshape(cfg.N, cfg.D)).astype(np.float32)
